# revision 32
# baseline (speedup 1.0000x reference)
"""Distributed Trainium2 (8 NeuronCores) kernel for nn_AdaptiveAttention.

Reference computation (b=2, n=2048, d=1024, 16 heads x 64):
    qkv = x @ W_qkv; q,k,v = split(qkv)
    attn = softmax(mask(q k^T / sqrt(dh)))
    out  = (attn @ v) @ W_out + b_out

Sharding: core c in [0,8) handles batch b = c//4 and head group g = c%4
(heads 4g..4g+3).  Data parallel over b, tensor parallel over heads.

Schedule (per core): pre-round q/k projections for the first head pair,
then 5 attention "rounds" over (head, i-half) combos.  Round 0 is solo
(head 0) and absorbs all remaining projections (v + second pair q/k);
rounds 1-3 run two heads concurrently -- their score matmuls land on
complementary PE row-tiles (0,0)/(64,0) since head_dim=64, so they
execute in parallel on the two halves of the 128x128 array.  Round 4 is
solo and absorbs the output projection of the first i-half.  All loops
are software-pipelined (scores/exp of iteration jc+1 are emitted before
attn@v of jc) so the in-order PE queue never waits on the Act/DVE
chain.

Unnormalized attention outputs plus softmax row sums (from a ones
column appended to v) are exchanged with four 8-rank AllToAll calls,
one per (head-pair, i-half), each fired as soon as its two head-rounds
finish so only the last call's latency is exposed.  Each rank sends
each peer only the 256 i-columns that peer owns.  Normalization
(reciprocal of the gathered sums, broadcast across the 64 head dims
with a tiny selector matmul) happens on the reader side, off the
critical path.  The output projection for i-half 0 overlaps round 4.

Numerics: all matmuls bf16 operands with fp32 PSUM accumulation.
Softmax runs without max-subtraction (scores are O(1) by construction)
as exp(s) * mask.
"""

import numpy as np
import ml_dtypes

import concourse.bass as bass
import concourse.tile as tile
from concourse import bacc, mybir
from concourse import bass_utils

BF16 = ml_dtypes.bfloat16

B = 2
N = 2048
D = 1024
HEADS = 16
HD = 64  # head dim
SCALE = HD ** -0.5
N_CORES = 8
HPC = 4  # heads per core
IB = 1024  # i-block size (one ib2 half)
NJ = N // 128  # 16 j-chunks

_cached_nc = None
_last_in_maps = None
_last_res = None


def _build():
    nc = bacc.Bacc("TRN2", target_bir_lowering=False, debug=False,
                   num_devices=N_CORES)

    f32 = mybir.dt.float32
    bf = mybir.dt.bfloat16

    fp8 = mybir.dt.float8e4
    xt = nc.dram_tensor("xt", [D, N], bf, kind="ExternalInput")
    wqkv = nc.dram_tensor("wqkv", [D, 768], bf, kind="ExternalInput")
    # mask is exactly 0/1 so it ships as fp8 (half the HBM traffic) and
    # the gpsimd software-DGE casts it to bf16 on the way into SBUF
    maskt = nc.dram_tensor("maskt", [N, N], fp8, kind="ExternalInput")
    wout = nc.dram_tensor("wout", [D, D], bf, kind="ExternalInput")
    e8 = nc.dram_tensor("e8", [8, 512], bf, kind="ExternalInput")
    out = nc.dram_tensor("out", [N // 4, D], bf, kind="ExternalOutput")

    with tile.TileContext(nc) as tc:
        with (
            tc.tile_pool(name="res", bufs=1) as res,
            tc.tile_pool(name="dram", bufs=1, space="DRAM") as dram,
            tc.tile_pool(name="pe", bufs=4) as p_e,
            tc.tile_pool(name="pp", bufs=4) as p_p,
            tc.tile_pool(name="pao", bufs=2) as pao,
            tc.tile_pool(name="pps", bufs=2, space="PSUM") as pp_s,
            tc.tile_pool(name="ppa1", bufs=1, space="PSUM") as pp_a1,
        ):
            # resident tensors
            # qkt: [qT01 | qT23 | kT01 | kT23], each [128, 2048] bf16
            qkt = res.tile([128, 4 * N], bf)
            # v_aug: per j-chunk jc block of 260 cols: 4x(64 v cols + ones)
            v_aug = res.tile([128, NJ * 260], bf)
            # mask, one tile per j-chunk so round 0 only waits on chunk 0
            mts = [res.tile([128, N], bf, name=f"mt{jc}") for jc in range(NJ)]
            wout_sb = res.tile([128, 8 * D], bf)
            e8_sb = res.tile([8, 512], bf)
            z65 = res.tile([128, 65], bf)  # zero lhsT for warm-keeper mms

            # AllToAll bounce buffers, one per (head, i-half) chunk --
            # small chunks fire at every round tail so only the very last
            # head's exchange sits in the kernel tail.  Shard j = rows
            # [65j, 65j+65) goes to rank j: the head's attention [64
            # rows] plus its softmax sums row; columns are the 256
            # i-cols that rank j owns within this i-half.
            a2a_ins = [[dram.tile([8 * 65, 256], bf, name=f"a2a_in{hl}{h}")
                        for h in range(2)] for hl in range(4)]
            a2a_outs = [[dram.tile([8 * 65, 256], bf,
                                   name=f"a2a_out{hl}{h}")
                         for h in range(2)] for hl in range(4)]

            nc.vector.memset(z65[:], 0.0)
            nc.vector.memset(v_aug[:], 1.0)

            # tiny warm-up AllToAll: absorbs the first-collective channel
            # setup cost (~25us) during the load/projection phase
            cwu_in = dram.tile([8, 16], bf, name="cwu_in")
            cwu_out = dram.tile([8, 16], bf, name="cwu_out")
            wu_sb = res.tile([8, 16], bf)
            nc.vector.memset(wu_sb[:], 0.0)
            nc.sync.dma_start(cwu_in[:, :], wu_sb[:])
            nc.gpsimd.collective_compute(
                "AllToAll", mybir.AluOpType.bypass,
                replica_groups=[[0, 1, 2, 3, 4, 5, 6, 7]],
                ins=[cwu_in[:].opt()], outs=[cwu_out[:].opt()],
            )

            # ---------------- phase 0: load + projections ----------------
            with (
                tc.tile_pool(name="ph0", bufs=1) as p0,
                tc.tile_pool(name="ppp", bufs=2, space="PSUM") as pp_p,
            ):
                xtr = p0.tile([128, 8 * N], bf)
                wr = p0.tile([128, 8 * 768], bf)
                # spread bulk loads over the three DMA-capable queues
                # (sync + scalar HWDGE, gpsimd SWDGE) so x/w aren't
                # bandwidth-starved; mask chunks queue behind them in
                # FIFO order, giving x/w priority
                for k in range(8):
                    eng = nc.sync if k % 2 == 0 else nc.scalar
                    eng.dma_start(xtr[:, N * k:N * (k + 1)],
                                  xt[128 * k:128 * (k + 1), :])
                    nc.gpsimd.dma_start(wr[:, 768 * k:768 * (k + 1)],
                                        wqkv[128 * k:128 * (k + 1), :])
                for jc in range(NJ):
                    nc.gpsimd.dma_start(
                        mts[jc][:], maskt[128 * jc:128 * (jc + 1), :])
                for k in range(8):
                    (nc.sync if k % 2 == 0 else nc.scalar).dma_start(
                        wout_sb[:, D * k:D * (k + 1)],
                        wout[128 * k:128 * (k + 1), :])
                nc.gpsimd.dma_start(e8_sb[:], e8[:, :])

                def proj_qk_group(t_i, nb):
                    wcol = 128 * t_i
                    ps = pp_p.tile([128, 512], f32, name="ps_qk", tag="pj")
                    for k in range(8):
                        nc.tensor.matmul(
                            ps[:],
                            wr[:, 768 * k + wcol:768 * k + wcol + 128],
                            xtr[:, N * k + 512 * nb:N * k + 512 * nb + 512],
                            start=(k == 0), stop=(k == 7),
                        )
                    nc.vector.tensor_copy(
                        qkt[:, N * t_i + 512 * nb:N * t_i + 512 * nb + 512],
                        ps[:])

                def proj_v_group(jc):
                    ps = pp_p.tile([128, 256], f32, name="ps_v", tag="pj")
                    for k in range(8):
                        nc.tensor.matmul(
                            ps[:],
                            xtr[:, N * k + 128 * jc:N * k + 128 * jc + 128],
                            wr[:, 768 * k + 512:768 * k + 768],
                            start=(k == 0), stop=(k == 7),
                        )
                    for h in range(4):
                        nc.vector.tensor_copy(
                            v_aug[:, 260 * jc + 65 * h:260 * jc + 65 * h + 64],
                            ps[:, 64 * h:64 * h + 64])

                # hl = head-local index (0..3) = 2*pair + hh
                def sc_iter(hl, ib2, jc):
                    pair, hh = hl // 2, hl % 2
                    q_off = N * pair
                    k_off = N * (2 + pair)
                    s_ps = pp_s.tile([128, IB], f32, name="s_ps", tag="mm")
                    for ih in range(2):
                        nc.tensor.matmul(
                            s_ps[:, 512 * ih:512 * ih + 512],
                            qkt[64 * hh:64 * hh + 64,
                                k_off + 128 * jc:k_off + 128 * jc + 128],
                            qkt[64 * hh:64 * hh + 64,
                                q_off + IB * ib2 + 512 * ih:
                                q_off + IB * ib2 + 512 * ih + 512],
                            start=True, stop=True,
                        )
                    return s_ps

                def ep_iter(ib2, jc, s_ps):
                    e_t = p_e.tile([128, IB], bf, name="e_t", tag="e_t")
                    nc.scalar.activation(
                        e_t[:], s_ps[:], mybir.ActivationFunctionType.Exp)
                    p_t = p_p.tile([128, IB], bf, name="p_t", tag="p_t")
                    nc.vector.tensor_mul(
                        p_t[:], e_t[:], mts[jc][:, IB * ib2:IB * ib2 + IB])
                    return p_t

                def av_iter(hl, jc, acc, p_t):
                    for ih in range(2):
                        nc.tensor.matmul(
                            acc[:, 512 * ih:512 * ih + 512],
                            v_aug[:, 260 * jc + 65 * hl:
                                  260 * jc + 65 * hl + 65],
                            p_t[:, 512 * ih:512 * ih + 512],
                            start=(jc == 0), stop=(jc == NJ - 1),
                        )

                def warm_mm(acc):
                    nc.tensor.matmul(
                        acc[:, 0:256], z65[:], v_aug[:, 0:256],
                        start=False, stop=False,
                        skip_group_check=True,
                    )

                def round_tail(hl, ib2, acc):
                    """Evacuate this head's raw attention output + sums
                    row, scatter into the AllToAll input shards (one
                    [65, 256] block per target rank), and fire the
                    exchange for this (head, i-half) chunk."""
                    ao = pao.tile([65, IB], bf, name="ao", tag="ao")
                    nc.vector.tensor_copy(ao[:], acc[:])
                    for j in range(8):
                        nc.sync.dma_start(
                            a2a_ins[hl][ib2][65 * j:65 * j + 65, :],
                            ao[:, 256 * (j % 4):256 * (j % 4) + 256])
                    nc.gpsimd.collective_compute(
                        "AllToAll",
                        mybir.AluOpType.bypass,
                        replica_groups=[[0, 1, 2, 3, 4, 5, 6, 7]],
                        ins=[a2a_ins[hl][ib2][:].opt()],
                        outs=[a2a_outs[hl][ib2][:].opt()],
                    )

                def new_acc(pool):
                    return pool.tile([65, IB], f32, name="acc", tag="acc")

                # pre-round: qT01 + kT01 so round 0 can start
                for nb in range(4):
                    proj_qk_group(0, nb)
                for nb in range(4):
                    proj_qk_group(2, nb)

                # ---- round 0: solo (h0, ib2=0) + all remaining proj ----
                acc0 = new_acc(pp_a1)
                proj_v_group(0)
                p_cur = ep_iter(0, 0, sc_iter(0, 0, 0))
                for jc in range(NJ):
                    if jc + 1 < NJ:
                        proj_v_group(jc + 1)
                        if jc % 2 == 0:
                            g_idx = jc // 2
                            proj_qk_group(1 if g_idx < 4 else 3, g_idx % 4)
                        p_nxt = ep_iter(0, jc + 1, sc_iter(0, 0, jc + 1))
                    av_iter(0, jc, acc0, p_cur)
                    p_cur = p_nxt
                round_tail(0, 0, acc0)

            # projections done: xtr/wr freed; open the second acc pool and
            # the phase-2 staging tiles
            with (
                tc.tile_pool(name="ppa2", bufs=1, space="PSUM") as pp_a2,
                tc.tile_pool(name="ph2", bufs=1) as p2,
                tc.tile_pool(name="ost", bufs=2) as po,
            ):
                # gathered attention [pair][g] and normalized copies,
                # plus gathered sums/reciprocals per pair, per i-half
                at_sb = [[[p2.tile([128, 256], bf, name=f"at{h}_{p}_{g}")
                           for g in range(4)] for p in range(2)]
                         for h in range(2)]
                at_n = [[[p2.tile([128, 256], bf, name=f"an{h}_{p}_{g}")
                          for g in range(4)] for p in range(2)]
                        for h in range(2)]
                sums_sb = [[p2.tile([8, 256], bf, name=f"sm{h}_{p}")
                            for p in range(2)] for h in range(2)]
                recs = [[p2.tile([8, 256], bf, name=f"rc{h}_{p}")
                         for p in range(2)] for h in range(2)]
                pid = nc.gpsimd.partition_id()
                goff = (pid // 4) * 260  # my batch group's a2a row base

                def emit_chunk_reads(hl, ib2, ms):
                    # per source rank g: attention block [64, 256] at row
                    # 65g, sums row at 65g + 64.  On the gpsimd queue
                    # (idle mid-kernel) so a slow collective can't
                    # head-of-line-block the sync queue; the wait_until
                    # stamp keeps the scheduler from hoisting the reads
                    # ahead of the preceding round's work.
                    pair, hh = hl // 2, hl % 2
                    src = a2a_outs[hl][ib2]
                    with tc.tile_wait_until(ms):
                        for g in range(4):
                            nc.gpsimd.dma_start(
                                at_sb[ib2][pair][g][64 * hh:64 * hh + 64, :],
                                src[bass.ds(goff + 65 * g, 64), :])
                        nc.gpsimd.dma_start(
                            sums_sb[ib2][pair][4 * hh:4 * hh + 4, :],
                            src[bass.ds(goff + 64, 4, 65), :])

                def emit_norm(pair, ib2, ms):
                    with tc.tile_wait_until(ms):
                        with nc.allow_low_precision(
                                reason="softmax recip bf16"):
                            nc.vector.reciprocal(recs[ib2][pair][:],
                                                 sums_sb[ib2][pair][:])

                def emit_bcmul(pair, ib2, g, ms):
                    with tc.tile_wait_until(ms):
                        bc = pp_s.tile([128, 256], f32, name="bc", tag="mm")
                        nc.tensor.matmul(bc[:],
                                         e8_sb[:, 128 * g:128 * g + 128],
                                         recs[ib2][pair][:], start=True,
                                         stop=True)
                        nc.vector.tensor_mul(at_n[ib2][pair][g][:],
                                             at_sb[ib2][pair][g][:], bc[:])

                def outproj_group(ib2, io, nh, ms):
                    with tc.tile_wait_until(ms):
                        ps = pp_s.tile([128, 512], f32, name="ps_o", tag="mm")
                        for ki, (p, g) in enumerate(
                                [(p, g) for g in range(4) for p in range(2)]):
                            nc.tensor.matmul(
                                ps[:],
                                at_n[ib2][p][g][:, 128 * io:128 * io + 128],
                                wout_sb[:, D * (2 * g + p) + 512 * nh:
                                        D * (2 * g + p) + 512 * nh + 512],
                                start=(ki == 0), stop=(ki == 7),
                            )
                        ot = po.tile([128, 512], bf, name="ot", tag="ot")
                        nc.vector.tensor_copy(ot[:], ps[:])
                        nc.sync.dma_start(
                            out[256 * ib2 + 128 * io:
                                256 * ib2 + 128 * io + 128,
                                512 * nh:512 * nh + 512],
                            ot[:])

                # ---- rounds 1-3: paired heads on complementary row-tiles
                # R1: (h1, ib0) + (h2, ib0)
                # R2: (h3, ib0) + (h0, ib1)
                # R3: (h1, ib1) + (h2, ib1)
                # each tail fires its own chunk; reads for chunks fired a
                # round earlier are emitted at the following round's end
                paired = [
                    ((1, 0), (2, 0), [(0, 0, 0.105)]),
                    ((3, 0), (0, 1), [(1, 0, 0.145), (2, 0, 0.150)]),
                    ((1, 1), (2, 1), [(3, 0, 0.175), (0, 1, 0.180)]),
                ]
                for (hlA, ibA), (hlB, ibB), rds in paired:
                    accA = new_acc(pp_a1)
                    accB = new_acc(pp_a2)
                    pA = ep_iter(ibA, 0, sc_iter(hlA, ibA, 0))
                    pB = ep_iter(ibB, 0, sc_iter(hlB, ibB, 0))
                    for jc in range(NJ):
                        if jc + 1 < NJ:
                            pA_n = ep_iter(ibA, jc + 1,
                                           sc_iter(hlA, ibA, jc + 1))
                            pB_n = ep_iter(ibB, jc + 1,
                                           sc_iter(hlB, ibB, jc + 1))
                        av_iter(hlA, jc, accA, pA)
                        av_iter(hlB, jc, accB, pB)
                        if jc not in (0, NJ - 1):
                            warm_mm(accA)
                        pA, pB = pA_n, pB_n
                    round_tail(hlA, ibA, accA)
                    round_tail(hlB, ibB, accB)
                    for rd in rds:
                        emit_chunk_reads(*rd)

                # ---- round 4: solo (h3, ib1) + normalize + output proj
                # of i-half 0 (both its chunks have been read by now)
                acc4 = new_acc(pp_a1)
                norm_jobs = [("recip", 0, 0), ("recip", 1, 0)]
                norm_jobs += [("bcmul", p, 0, g)
                              for p in range(2) for g in range(4)]
                op_jobs = [(0, io, nh) for io in range(2) for nh in range(2)]
                p_cur = ep_iter(1, 0, sc_iter(3, 1, 0))
                ms4 = 0.180
                for jc in range(NJ):
                    if jc + 1 < NJ:
                        p_nxt = ep_iter(1, jc + 1, sc_iter(3, 1, jc + 1))
                    av_iter(3, jc, acc4, p_cur)
                    if norm_jobs:
                        j = norm_jobs.pop(0)
                        if j[0] == "recip":
                            emit_norm(j[1], j[2], ms4)
                        else:
                            emit_bcmul(j[1], j[2], j[3], ms4)
                        ms4 += 0.002
                    elif jc % 3 == 1 and op_jobs:
                        outproj_group(*op_jobs.pop(0), ms4)
                        ms4 += 0.004
                    if jc not in (0, NJ - 1):
                        warm_mm(acc4)
                    p_cur = p_nxt
                round_tail(3, 1, acc4)
                for job in op_jobs:
                    outproj_group(*job, ms4)
                    ms4 += 0.004
                emit_chunk_reads(1, 1, 0.210)
                emit_chunk_reads(2, 1, 0.212)
                emit_chunk_reads(3, 1, 0.215)

                # ---- tail: normalize + project i-half 1
                for p in range(2):
                    emit_norm(p, 1, 0.220)
                for p in range(2):
                    for g in range(4):
                        emit_bcmul(p, 1, g, 0.221)
                for io in range(2):
                    for nh in range(2):
                        outproj_group(1, io, nh, 0.223)

    nc.compile()
    return nc


def _get_nc():
    global _cached_nc
    if _cached_nc is None:
        _cached_nc = _build()
    return _cached_nc


def kernel(x, mask, W_qkv, W_out, b_out):
    x = np.asarray(x, dtype=np.float32)
    mask = np.asarray(mask)
    W_qkv = np.asarray(W_qkv, dtype=np.float32)
    W_out = np.asarray(W_out, dtype=np.float32)
    b_out = np.asarray(b_out, dtype=np.float32)

    nc = _get_nc()

    FP8 = ml_dtypes.float8_e4m3
    maskt_fp8 = np.ascontiguousarray(mask.reshape(N, N).T).astype(FP8)
    wout_bf = W_out.astype(BF16)
    # normalization selector: e8[s, 128g + r] = 1 iff s == 4*(r//64) + g
    # (gathered sums live at partition 4hh + g)
    e8 = np.zeros((8, 512), dtype=np.float32)
    for g in range(4):
        for r in range(128):
            e8[4 * (r // 64) + g, 128 * g + r] = 1.0
    e8 = np.ascontiguousarray(e8).astype(BF16)

    in_maps = []
    for c in range(N_CORES):
        b = c // 4
        g = c % 4
        hs = slice(g * HPC * HD, (g + 1) * HPC * HD)  # 256 cols of this core
        wq = W_qkv[:, 0 * D:1 * D][:, hs] * np.float32(SCALE)
        wk = W_qkv[:, 1 * D:2 * D][:, hs]
        wv = W_qkv[:, 2 * D:3 * D][:, hs]
        wqkv_c = np.ascontiguousarray(
            np.concatenate([wq, wk, wv], axis=1)).astype(BF16)
        xt_c = np.ascontiguousarray(x[b].T).astype(BF16)
        in_maps.append({
            "xt": xt_c,
            "wqkv": wqkv_c,
            "maskt": maskt_fp8,
            "wout": wout_bf,
            "e8": e8,
        })

    global _last_in_maps, _last_res
    _last_in_maps = in_maps

    res = bass_utils.run_bass_kernel_spmd(
        nc, in_maps, core_ids=list(range(N_CORES)))
    _last_res = res

    out_full = np.empty((B, N, D), dtype=np.float32)
    for c in range(N_CORES):
        b = c // 4
        g = c % 4
        core_out = res.results[c]["out"].astype(np.float32)
        out_full[b, 256 * g:256 * g + 256, :] = core_out[0:256]
        out_full[b, 1024 + 256 * g:1024 + 256 * g + 256, :] = core_out[256:512]
    out_full += b_out
    return out_full


# revision 50
# speedup vs baseline: 1.0208x; 1.0208x over previous
"""Distributed Trainium2 (8 NeuronCores) kernel for nn_AdaptiveAttention.

Reference computation (b=2, n=2048, d=1024, 16 heads x 64):
    qkv = x @ W_qkv; q,k,v = split(qkv)
    attn = softmax(mask(q k^T / sqrt(dh)))
    out  = (attn @ v) @ W_out + b_out

Sharding: core c in [0,8) handles batch b = c//4 and head group g = c%4
(heads 4g..4g+3).  Data parallel over b, tensor parallel over heads.

Schedule (per core): a short pre-round projects q/k for the first head
pair, then FOUR fully-paired attention rounds cover the 8 (head,
i-half) combos; in every round the two concurrent heads' score matmuls
land on complementary PE row-tiles (0,0)/(64,0) since head_dim=64.
Remaining projections (v just-in-time, second-pair q/k) are woven into
rounds 0-1 using the shared score-PSUM pool, so both attn@v
accumulator pools coexist with it in exactly 8 PSUM banks.  Rounds are
paced by the Act engine (exp) at ~2.7us per j-chunk; PE rides along
with ~35% slack that absorbs the woven projections.

Each head-round's tail evacuates raw attention + softmax sums (ones
column appended to v) and fires a small per-(head, i-half) 8-rank
AllToAll sending each peer only the 256 i-columns it owns.  Chunks are
consumed a round later; normalization (reciprocal of gathered sums,
broadcast via a tiny selector matmul) and the output projection of
i-half 0 overlap round 3 and the final exchange's rank-skew window, so
the kernel tail holds only the last chunk's exchange plus i-half 1's
projection.  The mask ships as fp8 (exact for 0/1) and is cast to bf16
by the gpsimd software-DGE on the way into SBUF, halving its HBM
traffic.

Numerics: all matmuls bf16 operands with fp32 PSUM accumulation.
Softmax runs without max-subtraction (scores are O(1) by construction)
as exp(s) * mask.
"""

import numpy as np
import ml_dtypes

import concourse.bass as bass
import concourse.tile as tile
from concourse import bacc, mybir
from concourse import bass_utils

BF16 = ml_dtypes.bfloat16

B = 2
N = 2048
D = 1024
HEADS = 16
HD = 64  # head dim
SCALE = HD ** -0.5
N_CORES = 8
HPC = 4  # heads per core
IB = 1024  # i-block size (one ib2 half)
NJ = N // 128  # 16 j-chunks

_cached_nc = None
_last_in_maps = None
_last_res = None


def _build():
    nc = bacc.Bacc("TRN2", target_bir_lowering=False, debug=False,
                   num_devices=N_CORES)

    f32 = mybir.dt.float32
    bf = mybir.dt.bfloat16
    fp8 = mybir.dt.float8e4

    xt = nc.dram_tensor("xt", [D, N], bf, kind="ExternalInput")
    wqkv = nc.dram_tensor("wqkv", [D, 768], bf, kind="ExternalInput")
    # mask is exactly 0/1 so it ships as fp8 (half the HBM traffic) and
    # the gpsimd software-DGE casts it to bf16 on the way into SBUF
    maskt = nc.dram_tensor("maskt", [N, N], fp8, kind="ExternalInput")
    wout = nc.dram_tensor("wout", [D, D], bf, kind="ExternalInput")
    e8 = nc.dram_tensor("e8", [8, 512], bf, kind="ExternalInput")
    out = nc.dram_tensor("out", [N // 4, D], bf, kind="ExternalOutput")

    with tile.TileContext(nc) as tc:
        with (
            tc.tile_pool(name="res", bufs=1) as res,
            tc.tile_pool(name="dram", bufs=1, space="DRAM") as dram,
            tc.tile_pool(name="pe", bufs=4) as p_e,
            tc.tile_pool(name="pp", bufs=4) as p_p,
            tc.tile_pool(name="pao", bufs=2) as pao,
            tc.tile_pool(name="ph2", bufs=1) as p2,
            tc.tile_pool(name="ost", bufs=2) as po,
            tc.tile_pool(name="pps", bufs=2, space="PSUM") as pp_s,
            tc.tile_pool(name="ppa1", bufs=1, space="PSUM") as pp_a1,
            tc.tile_pool(name="ppa2", bufs=1, space="PSUM") as pp_a2,
        ):
            # resident tensors
            # qkt: [qT01 | qT23 | kT01 | kT23], each [128, 2048] bf16
            qkt = res.tile([128, 4 * N], bf)
            # v_aug: per j-chunk jc block of 260 cols: 4x(64 v cols + ones)
            v_aug = res.tile([128, NJ * 260], bf)
            # mask, one tile per j-chunk for fine-grained load deps
            mts = [res.tile([128, N], bf, name=f"mt{jc}") for jc in range(NJ)]
            wout_sb = res.tile([128, 8 * D], bf)
            e8_sb = res.tile([8, 512], bf)
            z65 = res.tile([128, 65], bf)  # zero lhsT for warm-keeper mms

            # AllToAll bounce buffers, one per (head, i-half) chunk --
            # small chunks fire at every round tail so only the very last
            # head's exchange sits in the kernel tail.  Shard j = rows
            # [65j, 65j+65) goes to rank j: the head's attention [64
            # rows] plus its softmax sums row; columns are the 256
            # i-cols that rank j owns within this i-half.
            a2a_ins = [[dram.tile([8 * 65, 256], bf, name=f"a2a_in{hl}{h}")
                        for h in range(2)] for hl in range(4)]
            a2a_outs = [[dram.tile([8 * 65, 256], bf,
                                   name=f"a2a_out{hl}{h}")
                         for h in range(2)] for hl in range(4)]

            # gathered attention [pair][g] / normalized copies / sums
            at_sb = [[[p2.tile([128, 256], bf, name=f"at{h}_{p}_{g}")
                       for g in range(4)] for p in range(2)]
                     for h in range(2)]
            at_n = [[[p2.tile([128, 256], bf, name=f"an{h}_{p}_{g}")
                      for g in range(4)] for p in range(2)]
                    for h in range(2)]
            sums_sb = [[p2.tile([8, 256], bf, name=f"sm{h}_{p}")
                        for p in range(2)] for h in range(2)]
            recs = [[p2.tile([8, 256], bf, name=f"rc{h}_{p}")
                     for p in range(2)] for h in range(2)]

            nc.vector.memset(z65[:], 0.0)
            nc.vector.memset(v_aug[:], 1.0)

            # tiny warm-up AllToAll: absorbs the first-collective channel
            # setup cost (~60us) during the load/projection phase
            cwu_in = dram.tile([8, 16], bf, name="cwu_in")
            cwu_out = dram.tile([8, 16], bf, name="cwu_out")
            wu_sb = res.tile([8, 16], bf)
            nc.vector.memset(wu_sb[:], 0.0)
            nc.sync.dma_start(cwu_in[:, :], wu_sb[:])
            nc.gpsimd.collective_compute(
                "AllToAll", mybir.AluOpType.bypass,
                replica_groups=[[0, 1, 2, 3, 4, 5, 6, 7]],
                ins=[cwu_in[:].opt()], outs=[cwu_out[:].opt()],
            )

            pid = nc.sync.partition_id()
            goff = (pid // 4) * 260  # my batch group's a2a row base

            with (
                tc.tile_pool(name="ph0", bufs=1) as p0,
            ):
                xtr = [p0.tile([128, N], bf, name=f"xtr{k}")
                       for k in range(8)]
                wr = [p0.tile([128, 768], bf, name=f"wr{k}")
                      for k in range(8)]
                # spread bulk loads over the three DMA-capable queues
                # (sync + scalar HWDGE, gpsimd SWDGE); mask/wout queue
                # behind x/w in FIFO order, giving x/w priority
                for k in range(8):
                    (nc.sync if k % 2 == 0 else nc.scalar).dma_start(
                        xtr[k][:], xt[128 * k:128 * (k + 1), :])
                    nc.gpsimd.dma_start(wr[k][:],
                                        wqkv[128 * k:128 * (k + 1), :])
                for jc in range(NJ):
                    nc.gpsimd.dma_start(
                        mts[jc][:], maskt[128 * jc:128 * (jc + 1), :])
                for k in range(8):
                    (nc.sync if k % 2 == 0 else nc.scalar).dma_start(
                        wout_sb[:, D * k:D * (k + 1)],
                        wout[128 * k:128 * (k + 1), :])
                nc.gpsimd.dma_start(e8_sb[:], e8[:, :])

                def proj_qk_group(t_i, nb):
                    wcol = 128 * t_i
                    ps = pp_s.tile([128, 512], f32, name="ps_qk", tag="mm")
                    for k in range(8):
                        nc.tensor.matmul(
                            ps[:],
                            wr[k][:, wcol:wcol + 128],
                            xtr[k][:, 512 * nb:512 * nb + 512],
                            start=(k == 0), stop=(k == 7),
                        )
                    nc.vector.tensor_copy(
                        qkt[:, N * t_i + 512 * nb:N * t_i + 512 * nb + 512],
                        ps[:])

                def proj_v_group(jc):
                    ps = pp_s.tile([128, 256], f32, name="ps_v", tag="mm")
                    for k in range(8):
                        nc.tensor.matmul(
                            ps[:],
                            xtr[k][:, 128 * jc:128 * jc + 128],
                            wr[k][:, 512:768],
                            start=(k == 0), stop=(k == 7),
                        )
                    for h in range(4):
                        nc.vector.tensor_copy(
                            v_aug[:, 260 * jc + 65 * h:260 * jc + 65 * h + 64],
                            ps[:, 64 * h:64 * h + 64])

                # hl = head-local index (0..3) = 2*pair + hh
                def sc_iter(hl, ib2, jc):
                    pair, hh = hl // 2, hl % 2
                    q_off = N * pair
                    k_off = N * (2 + pair)
                    s_ps = pp_s.tile([128, IB], f32, name="s_ps", tag="mm")
                    for ih in range(2):
                        nc.tensor.matmul(
                            s_ps[:, 512 * ih:512 * ih + 512],
                            qkt[64 * hh:64 * hh + 64,
                                k_off + 128 * jc:k_off + 128 * jc + 128],
                            qkt[64 * hh:64 * hh + 64,
                                q_off + IB * ib2 + 512 * ih:
                                q_off + IB * ib2 + 512 * ih + 512],
                            start=True, stop=True,
                        )
                    return s_ps

                def ep_iter(ib2, jc, s_ps):
                    e_t = p_e.tile([128, IB], bf, name="e_t", tag="e_t")
                    nc.scalar.activation(
                        e_t[:], s_ps[:], mybir.ActivationFunctionType.Exp)
                    p_t = p_p.tile([128, IB], bf, name="p_t", tag="p_t")
                    nc.vector.tensor_mul(
                        p_t[:], e_t[:], mts[jc][:, IB * ib2:IB * ib2 + IB])
                    return p_t

                def av_iter(hl, jc, acc, p_t):
                    for ih in range(2):
                        nc.tensor.matmul(
                            acc[:, 512 * ih:512 * ih + 512],
                            v_aug[:, 260 * jc + 65 * hl:
                                  260 * jc + 65 * hl + 65],
                            p_t[:, 512 * ih:512 * ih + 512],
                            start=(jc == 0), stop=(jc == NJ - 1),
                        )

                def warm_mm(acc):
                    nc.tensor.matmul(
                        acc[:, 0:256], z65[:], v_aug[:, 0:256],
                        start=False, stop=False,
                        skip_group_check=True,
                    )

                def round_tail(hl, ib2, acc):
                    """Evacuate this head's raw attention output + sums
                    row, scatter into the AllToAll input shards (one
                    [65, 256] block per target rank), and fire the
                    exchange for this (head, i-half) chunk."""
                    ao = pao.tile([65, IB], bf, name="ao", tag="ao")
                    nc.vector.tensor_copy(ao[:], acc[:])
                    for j in range(8):
                        nc.sync.dma_start(
                            a2a_ins[hl][ib2][65 * j:65 * j + 65, :],
                            ao[:, 256 * (j % 4):256 * (j % 4) + 256])
                    nc.gpsimd.collective_compute(
                        "AllToAll",
                        mybir.AluOpType.bypass,
                        replica_groups=[[0, 1, 2, 3, 4, 5, 6, 7]],
                        ins=[a2a_ins[hl][ib2][:].opt()],
                        outs=[a2a_outs[hl][ib2][:].opt()],
                    )

                def emit_chunk_reads(hl, ib2, ms):
                    # per source rank g: attention block [64, 256] at row
                    # 65g, sums row at 65g + 64.  Sync-queue DMAs (whose
                    # collective-completion waits are reliably enforced),
                    # emitted a full round after the chunk fires so the
                    # wait is short and can't head-of-line-block later
                    # tail writes; the wait_until stamp keeps the
                    # scheduler from hoisting them earlier.
                    pair, hh = hl // 2, hl % 2
                    src = a2a_outs[hl][ib2]
                    with tc.tile_wait_until(ms):
                        for g in range(4):
                            nc.sync.dma_start(
                                at_sb[ib2][pair][g][64 * hh:64 * hh + 64, :],
                                src[bass.ds(goff + 65 * g, 64), :])
                        nc.sync.dma_start(
                            sums_sb[ib2][pair][4 * hh:4 * hh + 4, :],
                            src[bass.ds(goff + 64, 4, 65), :])

                def emit_norm(pair, ib2, ms):
                    with tc.tile_wait_until(ms):
                        with nc.allow_low_precision(
                                reason="softmax recip bf16"):
                            nc.vector.reciprocal(recs[ib2][pair][:],
                                                 sums_sb[ib2][pair][:])

                def emit_bcmul(pair, ib2, g, ms):
                    with tc.tile_wait_until(ms):
                        bc = pp_s.tile([128, 256], f32, name="bc", tag="mm")
                        nc.tensor.matmul(bc[:],
                                         e8_sb[:, 128 * g:128 * g + 128],
                                         recs[ib2][pair][:], start=True,
                                         stop=True)
                        nc.vector.tensor_mul(at_n[ib2][pair][g][:],
                                             at_sb[ib2][pair][g][:], bc[:])

                def outproj_group(ib2, io, nh, ms):
                    with tc.tile_wait_until(ms):
                        ps = pp_s.tile([128, 512], f32, name="ps_o",
                                       tag="mm")
                        for ki, (p, g) in enumerate(
                                [(p, g) for g in range(4)
                                 for p in range(2)]):
                            nc.tensor.matmul(
                                ps[:],
                                at_n[ib2][p][g][:, 128 * io:128 * io + 128],
                                wout_sb[:, D * (2 * g + p) + 512 * nh:
                                        D * (2 * g + p) + 512 * nh + 512],
                                start=(ki == 0), stop=(ki == 7),
                            )
                        ot = po.tile([128, 512], bf, name="ot", tag="ot")
                        nc.vector.tensor_copy(ot[:], ps[:])
                        nc.sync.dma_start(
                            out[256 * ib2 + 128 * io:
                                256 * ib2 + 128 * io + 128,
                                512 * nh:512 * nh + 512],
                            ot[:])

                def new_acc(pool):
                    return pool.tile([65, IB], f32, name="acc", tag="acc")

                # pre-round: qT01 + kT01 + the first halves of qT23/kT23
                # that R1's h2 consumes from jc 0 (both pair-1 heads
                # contract against kT23!) + first v chunk
                for nb in range(4):
                    proj_qk_group(0, nb)
                for nb in range(4):
                    proj_qk_group(2, nb)
                proj_qk_group(1, 0)
                proj_qk_group(1, 1)
                proj_qk_group(3, 0)
                proj_qk_group(3, 1)
                proj_v_group(0)

                # ---- four fully-paired rounds; weave lists give each
                # round's extra PE work as (jc -> thunk) slots
                def run_round(hlA, ibA, hlB, ibB, weave, reads, tail_jobs):
                    accA = new_acc(pp_a1)
                    accB = new_acc(pp_a2)
                    pA = ep_iter(ibA, 0, sc_iter(hlA, ibA, 0))
                    pB = ep_iter(ibB, 0, sc_iter(hlB, ibB, 0))
                    for jc in range(NJ):
                        if jc + 1 < NJ:
                            pA_n = ep_iter(ibA, jc + 1,
                                           sc_iter(hlA, ibA, jc + 1))
                            pB_n = ep_iter(ibB, jc + 1,
                                           sc_iter(hlB, ibB, jc + 1))
                        av_iter(hlA, jc, accA, pA)
                        av_iter(hlB, jc, accB, pB)
                        for job in weave.get(jc, ()):
                            job()
                        if jc not in (0, NJ - 1):
                            warm_mm(accA)
                        pA, pB = pA_n, pB_n
                    round_tail(hlA, ibA, accA)
                    round_tail(hlB, ibB, accB)
                    for rd in reads:
                        emit_chunk_reads(*rd)
                    for job in tail_jobs:
                        job()

                # R0: (h0, ib0) + (h1, ib0); weave v just-in-time
                w0 = {jc: [lambda jc=jc: proj_v_group(jc + 1)]
                      for jc in range(NJ - 1)}
                run_round(0, 0, 1, 0, w0, [], [])

                # R1: (h2, ib0) + (h1, ib1); weave the back halves of
                # kT23 (h2 reads nb2 at jc8, nb3 at jc12 -- woven well
                # before) and qT23 (first used by R3)
                w1 = {2 * i + 1: [lambda t=t, nb=nb: proj_qk_group(t, nb)]
                      for i, (t, nb) in enumerate(
                          [(3, 2), (3, 3), (1, 2), (1, 3)])}
                run_round(2, 0, 1, 1, w1,
                          [(0, 0, 0.105), (1, 0, 0.107)], [])

            # projections done: xtr/wr freed
            # R2: (h3, ib0) + (h0, ib1)
            run_round(3, 0, 0, 1, {},
                      [(2, 0, 0.148), (1, 1, 0.150)], [])

            # R3: (h2, ib1) + (h3, ib1); weave i-half-0 normalization;
            # its output projection fills the final exchange's rank-skew
            # window in the tail
            run_round(2, 1, 3, 1, {},
                      [(3, 0, 0.190), (0, 1, 0.192)], [])

            # ---- tail: read + normalize + project both i-halves
            for p in range(2):
                emit_norm(p, 0, 0.194)
            for p in range(2):
                for g in range(4):
                    emit_bcmul(p, 0, g, 0.195)
            for io in range(2):
                for nh in range(2):
                    outproj_group(0, io, nh, 0.197)
            emit_chunk_reads(2, 1, 0.200)
            emit_chunk_reads(3, 1, 0.203)
            for p in range(2):
                emit_norm(p, 1, 0.206)
            for p in range(2):
                for g in range(4):
                    emit_bcmul(p, 1, g, 0.207)
            for io in range(2):
                for nh in range(2):
                    outproj_group(1, io, nh, 0.209)

    nc.compile()
    return nc


def _get_nc():
    global _cached_nc
    if _cached_nc is None:
        _cached_nc = _build()
    return _cached_nc


def kernel(x, mask, W_qkv, W_out, b_out):
    x = np.asarray(x, dtype=np.float32)
    mask = np.asarray(mask)
    W_qkv = np.asarray(W_qkv, dtype=np.float32)
    W_out = np.asarray(W_out, dtype=np.float32)
    b_out = np.asarray(b_out, dtype=np.float32)

    nc = _get_nc()

    FP8 = ml_dtypes.float8_e4m3
    maskt_fp8 = np.ascontiguousarray(mask.reshape(N, N).T).astype(FP8)
    wout_bf = W_out.astype(BF16)
    # normalization selector: e8[s, 128g + r] = 1 iff s == 4*(r//64) + g
    # (gathered sums live at partition 4hh + g)
    e8 = np.zeros((8, 512), dtype=np.float32)
    for g in range(4):
        for r in range(128):
            e8[4 * (r // 64) + g, 128 * g + r] = 1.0
    e8 = np.ascontiguousarray(e8).astype(BF16)

    in_maps = []
    for c in range(N_CORES):
        b = c // 4
        g = c % 4
        hs = slice(g * HPC * HD, (g + 1) * HPC * HD)  # 256 cols of this core
        wq = W_qkv[:, 0 * D:1 * D][:, hs] * np.float32(SCALE)
        wk = W_qkv[:, 1 * D:2 * D][:, hs]
        wv = W_qkv[:, 2 * D:3 * D][:, hs]
        wqkv_c = np.ascontiguousarray(
            np.concatenate([wq, wk, wv], axis=1)).astype(BF16)
        xt_c = np.ascontiguousarray(x[b].T).astype(BF16)
        in_maps.append({
            "xt": xt_c,
            "wqkv": wqkv_c,
            "maskt": maskt_fp8,
            "wout": wout_bf,
            "e8": e8,
        })

    global _last_in_maps, _last_res
    _last_in_maps = in_maps

    res = bass_utils.run_bass_kernel_spmd(
        nc, in_maps, core_ids=list(range(N_CORES)))
    _last_res = res

    out_full = np.empty((B, N, D), dtype=np.float32)
    for c in range(N_CORES):
        b = c // 4
        g = c % 4
        core_out = res.results[c]["out"].astype(np.float32)
        out_full[b, 256 * g:256 * g + 256, :] = core_out[0:256]
        out_full[b, 1024 + 256 * g:1024 + 256 * g + 256, :] = core_out[256:512]
    out_full += b_out
    return out_full
